# revision 5
# baseline (speedup 1.0000x reference)
"""MinEntropyConsensusLoss Trainium2 kernel.

ce = 0.5 * mean_b( min_c( -log_softmax(x)[b,c] - log_softmax(y)[b,c] ) )
   = 0.5 * mean_b( lse(x_b) + lse(y_b) - max_c(x[b,c] + y[b,c]) )

Data-parallel over 8 NeuronCores: each core streams 16384 rows of x and y,
computes per-row sum(exp(x)), sum(exp(y)) (ScalarE activation with fused
accumulate) and max(x+y) (VectorE tensor_tensor_reduce, fused add+max),
and writes the three [128, 128] stat tiles back. Host applies log and the
global mean in float64.
"""

import sys

sys.path.insert(0, "/opt/trn_rl_repo")

import numpy as np

import concourse.bacc as bacc
import concourse.mybir as mybir
import concourse.tile as tile
from concourse.bass_utils import run_bass_kernel_spmd

B, C = 131072, 256
N_CORES = 8
R = B // N_CORES          # rows per core = 16384
T = 16                    # rows per partition per chunk
CHUNK_ROWS = 128 * T      # 2048 rows per chunk (2 MiB per tensor)
NCH = R // CHUNK_ROWS     # 8 chunks
NG = NCH * T              # 128 row-groups per core

_cache = {}


def _build_nc(repeat=1):
    f32 = mybir.dt.float32
    nc = bacc.Bacc("TRN2", target_bir_lowering=False, debug=False)
    x_d = nc.dram_tensor("x", [R, C], f32, kind="ExternalInput")
    y_d = nc.dram_tensor("y", [R, C], f32, kind="ExternalInput")
    sx_d = nc.dram_tensor("sx", [128, NG], f32, kind="ExternalOutput")
    sy_d = nc.dram_tensor("sy", [128, NG], f32, kind="ExternalOutput")
    mxy_d = nc.dram_tensor("mxy", [128, NG], f32, kind="ExternalOutput")

    # chunk c, partition p holds rows c*CHUNK_ROWS + p*T + t  (t contiguous)
    x_v = x_d.ap().rearrange("(c p t) f -> c p (t f)", c=NCH, p=128, t=T)
    y_v = y_d.ap().rearrange("(c p t) f -> c p (t f)", c=NCH, p=128, t=T)

    with tile.TileContext(nc) as tc:
        with (
            tc.tile_pool(name="xin", bufs=2) as xin_pool,
            tc.tile_pool(name="yin", bufs=2) as yin_pool,
            tc.tile_pool(name="exps", bufs=2, space="PSUM") as exp_pool,
            tc.tile_pool(name="xys", bufs=2) as xy_pool,
            tc.tile_pool(name="stats", bufs=1) as stats_pool,
        ):
            sx_t = stats_pool.tile([128, NG], f32, tag="sx")
            sy_t = stats_pool.tile([128, NG], f32, tag="sy")
            mxy_t = stats_pool.tile([128, NG], f32, tag="mxy")

            for c in [c for _ in range(repeat) for c in range(NCH)]:
                x_t = xin_pool.tile([128, T * C], f32, tag="x")
                nc.sync.dma_start(x_t[:], x_v[c])
                y_t = yin_pool.tile([128, T * C], f32, tag="y")
                nc.sync.dma_start(y_t[:], y_v[c])

                for t in range(T):
                    g = c * T + t
                    sl = slice(t * C, (t + 1) * C)
                    ex = exp_pool.tile([128, C], f32, tag="ex")
                    nc.scalar.activation(
                        ex[:], x_t[:, sl], mybir.ActivationFunctionType.Exp,
                        accum_out=sx_t[:, g : g + 1],
                    )
                    ey = exp_pool.tile([128, C], f32, tag="ey")
                    nc.scalar.activation(
                        ey[:], y_t[:, sl], mybir.ActivationFunctionType.Exp,
                        accum_out=sy_t[:, g : g + 1],
                    )
                    xy = xy_pool.tile([128, C], f32, tag="xy")
                    nc.vector.tensor_tensor(
                        out=xy[:], in0=x_t[:, sl], in1=y_t[:, sl],
                        op=mybir.AluOpType.add,
                    )
                    dead = xy_pool.tile([128, C], f32, tag="dead")
                    nc.vector.tensor_scalar(
                        out=dead[:], in0=xy[:], scalar1=0.0, scalar2=-1.0e30,
                        op0=mybir.AluOpType.add, op1=mybir.AluOpType.max,
                        accum_out=mxy_t[:, g : g + 1],
                    )

            nc.sync.dma_start(sx_d.ap(), sx_t[:])
            nc.sync.dma_start(sy_d.ap(), sy_t[:])
            nc.sync.dma_start(mxy_d.ap(), mxy_t[:])

    nc.compile()
    return nc


def get_nc():
    if "nc" not in _cache:
        _cache["nc"] = _build_nc()
    return _cache["nc"]


def run_cores(x, y, trace=False, **kw):
    nc = get_nc()
    x = np.ascontiguousarray(np.asarray(x, dtype=np.float32))
    y = np.ascontiguousarray(np.asarray(y, dtype=np.float32))
    in_maps = [
        {"x": x[k * R : (k + 1) * R], "y": y[k * R : (k + 1) * R]}
        for k in range(N_CORES)
    ]
    return run_bass_kernel_spmd(nc, in_maps, list(range(N_CORES)), trace=trace, **kw)


def kernel(x, y):
    res = run_cores(x, y)
    total = 0.0
    for r in res.results:
        sx = r["sx"].astype(np.float64)
        sy = r["sy"].astype(np.float64)
        mxy = r["mxy"].astype(np.float64)
        total += float(np.sum(np.log(sx) + np.log(sy) - mxy))
    return np.float32(0.5 * total / B)


# revision 6
# speedup vs baseline: 1.8371x; 1.8371x over previous
"""MinEntropyConsensusLoss Trainium2 kernel.

ce = 0.5 * mean_b( min_c( -log_softmax(x)[b,c] - log_softmax(y)[b,c] ) )
   = 0.5 * mean_b( lse(x_b) + lse(y_b) - max_c(x[b,c] + y[b,c]) )

Data-parallel over 8 NeuronCores: each core streams 16384 rows of x and y,
computes per-row sum(exp(x)), sum(exp(y)) (ScalarE activation with fused
accumulate) and max(x+y) (VectorE tensor_tensor_reduce, fused add+max),
and writes the three [128, 128] stat tiles back. Host applies log and the
global mean in float64.
"""

import sys

sys.path.insert(0, "/opt/trn_rl_repo")

import numpy as np

import concourse.bacc as bacc
import concourse.mybir as mybir
import concourse.tile as tile
from concourse.bass_utils import run_bass_kernel_spmd

B, C = 131072, 256
N_CORES = 8
R = B // N_CORES          # rows per core = 16384
T = 16                    # rows per partition per chunk
CHUNK_ROWS = 128 * T      # 2048 rows per chunk (2 MiB per tensor)
NCH = R // CHUNK_ROWS     # 8 chunks
NG = NCH * T              # 128 row-groups per core

_cache = {}


def _build_nc(repeat=1):
    f32 = mybir.dt.float32
    nc = bacc.Bacc("TRN2", target_bir_lowering=False, debug=False)
    x_d = nc.dram_tensor("x", [R, C], f32, kind="ExternalInput")
    y_d = nc.dram_tensor("y", [R, C], f32, kind="ExternalInput")
    sx_d = nc.dram_tensor("sx", [128, NG], f32, kind="ExternalOutput")
    sy_d = nc.dram_tensor("sy", [128, NG], f32, kind="ExternalOutput")
    mxy_d = nc.dram_tensor("mxy", [128, NG], f32, kind="ExternalOutput")

    # chunk c, partition p holds rows c*CHUNK_ROWS + p*T + t  (t contiguous)
    x_v = x_d.ap().rearrange("(c p t) f -> c p (t f)", c=NCH, p=128, t=T)
    y_v = y_d.ap().rearrange("(c p t) f -> c p (t f)", c=NCH, p=128, t=T)

    with tile.TileContext(nc) as tc:
        with (
            tc.tile_pool(name="xin", bufs=2) as xin_pool,
            tc.tile_pool(name="yin", bufs=2) as yin_pool,
            tc.tile_pool(name="exps", bufs=2, space="PSUM") as exp_pool,
            tc.tile_pool(name="xys", bufs=2) as xy_pool,
            tc.tile_pool(name="stats", bufs=1) as stats_pool,
        ):
            sx_t = stats_pool.tile([128, NG], f32, tag="sx")
            sy_t = stats_pool.tile([128, NG], f32, tag="sy")
            mxy_t = stats_pool.tile([128, NG], f32, tag="mxy")

            def one_pass():
                for c in range(NCH):
                    x_t = xin_pool.tile([128, T * C], f32, tag="x")
                    nc.sync.dma_start(x_t[:], x_v[c])
                    y_t = yin_pool.tile([128, T * C], f32, tag="y")
                    nc.sync.dma_start(y_t[:], y_v[c])

                    for t in range(T):
                        g = c * T + t
                        sl = slice(t * C, (t + 1) * C)
                        ex = exp_pool.tile([128, C], f32, tag="ex")
                        nc.scalar.activation(
                            ex[:], x_t[:, sl], mybir.ActivationFunctionType.Exp,
                            accum_out=sx_t[:, g : g + 1],
                        )
                        ey = exp_pool.tile([128, C], f32, tag="ey")
                        nc.scalar.activation(
                            ey[:], y_t[:, sl], mybir.ActivationFunctionType.Exp,
                            accum_out=sy_t[:, g : g + 1],
                        )
                        xy = xy_pool.tile([128, C], f32, tag="xy")
                        nc.vector.tensor_tensor(
                            out=xy[:], in0=x_t[:, sl], in1=y_t[:, sl],
                            op=mybir.AluOpType.add,
                        )
                        dead = xy_pool.tile([128, C], f32, tag="dead")
                        nc.vector.tensor_scalar(
                            out=dead[:], in0=xy[:], scalar1=0.0, scalar2=-1.0e30,
                            op0=mybir.AluOpType.add, op1=mybir.AluOpType.max,
                            accum_out=mxy_t[:, g : g + 1],
                        )

            if repeat > 1:
                with tc.For_i(0, repeat, 1):
                    one_pass()
            else:
                one_pass()

            nc.sync.dma_start(sx_d.ap(), sx_t[:])
            nc.sync.dma_start(sy_d.ap(), sy_t[:])
            nc.sync.dma_start(mxy_d.ap(), mxy_t[:])

    nc.compile()
    return nc


def get_nc():
    if "nc" not in _cache:
        _cache["nc"] = _build_nc()
    return _cache["nc"]


def run_cores(x, y, trace=False, **kw):
    nc = get_nc()
    x = np.ascontiguousarray(np.asarray(x, dtype=np.float32))
    y = np.ascontiguousarray(np.asarray(y, dtype=np.float32))
    in_maps = [
        {"x": x[k * R : (k + 1) * R], "y": y[k * R : (k + 1) * R]}
        for k in range(N_CORES)
    ]
    return run_bass_kernel_spmd(nc, in_maps, list(range(N_CORES)), trace=trace, **kw)


def kernel(x, y):
    res = run_cores(x, y)
    total = 0.0
    for r in res.results:
        sx = r["sx"].astype(np.float64)
        sy = r["sy"].astype(np.float64)
        mxy = r["mxy"].astype(np.float64)
        total += float(np.sum(np.log(sx) + np.log(sy) - mxy))
    return np.float32(0.5 * total / B)


# revision 7
# speedup vs baseline: 2.5134x; 1.3682x over previous
"""MinEntropyConsensusLoss Trainium2 kernel.

ce = 0.5 * mean_b( min_c( -log_softmax(x)[b,c] - log_softmax(y)[b,c] ) )
   = 0.5 * mean_b( lse(x_b) + lse(y_b) - max_c(x[b,c] + y[b,c]) )

Data-parallel over 8 NeuronCores; each streams 16384 rows of x and y and
emits per-row stats: sum(exp(x)), sum(exp(y)) and max(x+y). Host applies
log and the global mean in float64 (permutation-invariant, so row->slot
mapping never needs to be undone).

Engine split (measured per 128-row group, DMA budget ~710ns):
  ACT    exp(y) in 2048-col batches; exp(x) half solo+fused-accum (row
         sums), half batched                      ~640ns
  GPSIMD x+y elementwise add, full 4096-col chunk ~700ns
  DVE    3D-batched reduce_max(x+y) + reduce_sum(exp) into stat tiles
                                                  ~670ns
"""

import sys

sys.path.insert(0, "/opt/trn_rl_repo")

import numpy as np

import concourse.bacc as bacc
import concourse.mybir as mybir
import concourse.tile as tile
from concourse.bass_utils import run_bass_kernel_spmd

B, C = 131072, 256
N_CORES = 8
R = B // N_CORES          # rows per core = 16384
T = 16                    # rows per partition per chunk
CHUNK_ROWS = 128 * T      # 2048 rows per chunk (2 MiB per tensor)
NCH = R // CHUNK_ROWS     # 8 chunks
NG = NCH * T              # 128 row-groups per core
NACC = T // 2             # groups per chunk whose sum(exp(x)) uses ACT accum

_cache = {}


def _build_nc(repeat=1):
    f32 = mybir.dt.float32
    A = mybir.AluOpType
    Exp = mybir.ActivationFunctionType.Exp
    X = mybir.AxisListType.X
    nc = bacc.Bacc("TRN2", target_bir_lowering=False, debug=False)
    x_d = nc.dram_tensor("x", [R, C], f32, kind="ExternalInput")
    y_d = nc.dram_tensor("y", [R, C], f32, kind="ExternalInput")
    sx_d = nc.dram_tensor("sx", [128, NG], f32, kind="ExternalOutput")
    sy_d = nc.dram_tensor("sy", [128, NG], f32, kind="ExternalOutput")
    mxy_d = nc.dram_tensor("mxy", [128, NG], f32, kind="ExternalOutput")

    # chunk c, partition p holds rows c*CHUNK_ROWS + p*T + t  (t contiguous)
    x_v = x_d.ap().rearrange("(c p t) f -> c p (t f)", c=NCH, p=128, t=T)
    y_v = y_d.ap().rearrange("(c p t) f -> c p (t f)", c=NCH, p=128, t=T)

    with tile.TileContext(nc) as tc:
        with (
            tc.tile_pool(name="xin", bufs=2) as xin_pool,
            tc.tile_pool(name="yin", bufs=2) as yin_pool,
            tc.tile_pool(name="expy", bufs=2) as expy_pool,
            tc.tile_pool(name="expx", bufs=2) as expx_pool,
            tc.tile_pool(name="xys", bufs=2) as xy_pool,
            tc.tile_pool(name="dead", bufs=2, space="PSUM") as dead_pool,
            tc.tile_pool(name="stats", bufs=1) as stats_pool,
        ):
            sx_t = stats_pool.tile([128, NG], f32, tag="sx")
            sy_t = stats_pool.tile([128, NG], f32, tag="sy")
            mxy_t = stats_pool.tile([128, NG], f32, tag="mxy")

            def one_pass():
                for c in range(NCH):
                    g0 = c * T
                    x_t = xin_pool.tile([128, T * C], f32, tag="x")
                    nc.sync.dma_start(x_t[:], x_v[c])
                    y_t = yin_pool.tile([128, T * C], f32, tag="y")
                    nc.sync.dma_start(y_t[:], y_v[c])

                    # --- ScalarE: exponentials ---
                    # x, groups 0..NACC-1: solo instructions with fused
                    # row-sum accumulate (main out is a dead store in PSUM)
                    for t in range(NACC):
                        dead = dead_pool.tile([128, C], f32, tag="dead")
                        nc.scalar.activation(
                            dead[:], x_t[:, t * C : (t + 1) * C], Exp,
                            accum_out=sx_t[:, g0 + t : g0 + t + 1],
                        )
                    # x, groups NACC..T-1: one batched exp; sums via DVE
                    ex = expx_pool.tile([128, (T - NACC) * C], f32, tag="ex")
                    nc.scalar.activation(ex[:], x_t[:, NACC * C :], Exp)
                    # y: batched exp in 2048-col instructions; sums via DVE
                    ey = expy_pool.tile([128, T * C], f32, tag="ey")
                    half = T * C // 2
                    nc.scalar.activation(ey[:, :half], y_t[:, :half], Exp)
                    nc.scalar.activation(ey[:, half:], y_t[:, half:], Exp)

                    # --- GPSIMD: x+y for the whole chunk ---
                    xy = xy_pool.tile([128, T * C], f32, tag="xy")
                    nc.gpsimd.tensor_tensor(
                        out=xy[:], in0=x_t[:], in1=y_t[:], op=A.add
                    )

                    # --- DVE: batched row reductions into stat tiles ---
                    nc.vector.reduce_max(
                        mxy_t[:, g0 : g0 + T],
                        xy[:].rearrange("p (t f) -> p t f", t=T),
                        axis=X,
                    )
                    nc.vector.reduce_sum(
                        sx_t[:, g0 + NACC : g0 + T],
                        ex[:].rearrange("p (t f) -> p t f", t=T - NACC),
                        axis=X,
                    )
                    nc.vector.reduce_sum(
                        sy_t[:, g0 : g0 + T],
                        ey[:].rearrange("p (t f) -> p t f", t=T),
                        axis=X,
                    )

            if repeat > 1:
                with tc.For_i(0, repeat, 1):
                    one_pass()
            else:
                one_pass()

            nc.sync.dma_start(sx_d.ap(), sx_t[:])
            nc.sync.dma_start(sy_d.ap(), sy_t[:])
            nc.sync.dma_start(mxy_d.ap(), mxy_t[:])

    nc.compile()
    return nc


def get_nc():
    if "nc" not in _cache:
        _cache["nc"] = _build_nc()
    return _cache["nc"]


def run_cores(x, y, trace=False, **kw):
    nc = get_nc()
    x = np.ascontiguousarray(np.asarray(x, dtype=np.float32))
    y = np.ascontiguousarray(np.asarray(y, dtype=np.float32))
    in_maps = [
        {"x": x[k * R : (k + 1) * R], "y": y[k * R : (k + 1) * R]}
        for k in range(N_CORES)
    ]
    return run_bass_kernel_spmd(nc, in_maps, list(range(N_CORES)), trace=trace, **kw)


def kernel(x, y):
    res = run_cores(x, y)
    total = 0.0
    for r in res.results:
        sx = r["sx"].astype(np.float64)
        sy = r["sy"].astype(np.float64)
        mxy = r["mxy"].astype(np.float64)
        total += float(np.sum(np.log(sx) + np.log(sy) - mxy))
    return np.float32(0.5 * total / B)
